# revision 76
# baseline (speedup 1.0000x reference)
"""Trainium2 Bass kernel for nn_MultiHeadMLPAttentionModel.

Model: per (b, n) point: pairwise = [radar_b(4), pt(2)] (radar constant over n).
  h1 = relu(pairwise @ enc_w1 + enc_b1)            [B,N,64]
  pf = h1 @ enc_w2 + enc_b2                        [B,N,64]
  sh = relu(einsum('bnf,hfd', pairwise, sc_w1) + sc_b1)
  logits = einsum('bnhd,hd', sh, sc_w2) + sc_b2    [B,N,4]
  w = softmax(logits, axis=n)
  ctx = einsum('bnh,bnd', w, pf)  -> out MLP -> [B]

Key algebraic restructurings used here:
  * pooling commutes with the (linear) second encoder layer since softmax
    weights sum to 1:  ctx = (sum_n w * h1) @ enc_w2 + enc_b2.  This removes
    the N-scale enc2 matmul entirely.
  * sc_b2 is constant over n, so it drops out of the softmax.
  * the radar part of pairwise is constant over n, so all layer-1 radar
    contributions fold into per-b bias vectors (computed on host: ~200 KFLOP
    of the model's 13 GFLOP).
  * softmax is computed without max-subtraction (logits are O(1) for this
    model; exp is evaluated in fp32) and normalization is deferred: the
    pooling matmul accumulates unnormalized sum_n exp(l)*h1 plus sum_n exp(l)
    (via an appended ones column), and the division happens once per b.

Sharding: pure data parallel over B: 8 cores x 16 rows each.  One SPMD Bass
program; per-core inputs differ only in data.
"""

import numpy as np

import concourse.bass as bass
import concourse.tile as tile
from concourse import bacc, mybir

B, N, HID, HEADS = 128, 8192, 64, 4
NCORES = 8
BPC = B // NCORES  # 16 batch rows per core
CHUNK = 512
NCH = N // CHUNK  # 16
NB = N // 128  # 64 point-blocks of 128

F32 = mybir.dt.float32
BF16 = mybir.dt.bfloat16
AF = mybir.ActivationFunctionType
ALU = mybir.AluOpType


def build_nc(reps=1, phases="ATPD"):
    from contextlib import ExitStack

    nc = bacc.Bacc()
    f32 = F32

    xp_d = nc.dram_tensor("xp", [BPC, 6, N], BF16, kind="ExternalInput")
    xpr_d = nc.dram_tensor("xpr", [NCH, 128, CHUNK], BF16, kind="ExternalInput")
    wbd_d = nc.dram_tensor("wbd", [128, 8 * 128], BF16, kind="ExternalInput")
    w2r_d = nc.dram_tensor("w2r", [128, 8 * 32], BF16, kind="ExternalInput")
    wenm_d = nc.dram_tensor("wenm", [6, BPC * 65], BF16, kind="ExternalInput")
    ew2b_d = nc.dram_tensor("ew2b", [65, 64], f32, kind="ExternalInput")
    ow1_d = nc.dram_tensor("ow1", [64, 256], f32, kind="ExternalInput")
    ob1_d = nc.dram_tensor("ob1", [1, 64], f32, kind="ExternalInput")
    w2o_d = nc.dram_tensor("w2o", [65, 1], f32, kind="ExternalInput")
    id128_d = nc.dram_tensor("id128", [128, 128], BF16, kind="ExternalInput")
    id64f_d = nc.dram_tensor("id64f", [64, 64], f32, kind="ExternalInput")
    sel4_d = nc.dram_tensor("sel4", [128, 16], f32, kind="ExternalInput")
    on16_d = nc.dram_tensor("on16", [1, BPC], f32, kind="ExternalInput")
    out_d = nc.dram_tensor("out", [BPC], f32, kind="ExternalOutput")

    with tile.TileContext(nc) as tc, ExitStack() as ctx:
        consts = ctx.enter_context(tc.tile_pool(name="consts", bufs=1))

        def cload(dram, shape, nm, dt=f32):
            t = consts.tile(shape, dt, name=nm, tag=nm)
            nc.sync.dma_start(t[:], dram[:])
            return t

        wbd_s = cload(wbd_d, [128, 8 * 128], "wbd_s", BF16)
        w2r_s = cload(w2r_d, [128, 8 * 32], "w2r_s", BF16)
        wenm_s = cload(wenm_d, [6, BPC * 65], "wenm_s", BF16)
        ew2b_s = cload(ew2b_d, [65, 64], "ew2b_s")
        ow1_s = cload(ow1_d, [64, 256], "ow1_s")
        ob1_s = cload(ob1_d, [1, 64], "ob1_s")
        w2o_s = cload(w2o_d, [65, 1], "w2o_s")
        id128_s = cload(id128_d, [128, 128], "id128_s", BF16)
        id64f_s = cload(id64f_d, [64, 64], "id64f_s")
        sel4_s = cload(sel4_d, [128, 16], "sel4_s")
        on16_s = cload(on16_d, [1, BPC], "on16_s")

        # n-major exp(logits): block t occupies cols [t*128, (t+1)*128), within
        # a block: partition p = n offset, col = 32*(b%4) + 4*(b//4) + h
        enm = consts.tile([128, NB * 128], BF16, name="enm", tag="enm")
        ctxnT = consts.tile([65, 64], f32, name="ctxnT", tag="ctxnT")
        obuf = consts.tile([65, BPC], f32, name="obuf", tag="obuf")
        fct = consts.tile([64, 64], f32, name="fct", tag="fct")
        res = consts.tile([1, BPC], f32, name="res", tag="res")
        nc.vector.memset(ctxnT[64:65, :], 1.0)
        nc.vector.memset(obuf[64:65, :], 1.0)

        if "A" not in phases:
            nc.vector.memset(enm[:, 0:8], 0.0)
        for _rep in range(reps):
            _build_body(
                nc, tc, xp_d, xpr_d, out_d,
                wbd_s, w2r_s, wenm_s, ew2b_s, ow1_s,
                ob1_s, w2o_s, id128_s, id64f_s, sel4_s, on16_s,
                enm, ctxnT, obuf, fct, res, phases,
            )

    if not nc.is_finalized():
        nc.finalize()
    return nc


def _build_body(
    nc, tc, xp_d, xpr_d, out_d,
    wbd_s, w2r_s, wenm_s, ew2b_s, ow1_s,
    ob1_s, w2o_s, id128_s, id64f_s, sel4_s, on16_s,
    enm, ctxnT, obuf, fct, res, phases="ATPD",
):
    from contextlib import ExitStack

    f32 = F32
    if "A" in phases:
        # ---- Phase A: score-net hidden + logits ---------------------------
        # sh: block-diagonal packing, 4 b's per matmul (K=18: 16 data rows +
        # 2 bias ones-rows), 4 concurrent row-tiles (one per b%4 group).
        # Round r covers 32 hidden dims; psum tile (r,i) = [4 slots x 32
        # dims, 512 pts].  Evac = pure relu (bias folded into the matmul).
        # lg: per (r,i) one K=128 matmul contracts the whole evac'd tile
        # (block-diag w2 slice), 4 concurrent col-tiles, accumulating over r.
        with ExitStack() as pctx:
            xpool = pctx.enter_context(tc.tile_pool(name="xpA", bufs=3))
            shpool = pctx.enter_context(tc.tile_pool(name="shp", bufs=34))
            epool = pctx.enter_context(tc.tile_pool(name="ep", bufs=2))
            psA = pctx.enter_context(tc.tile_pool(name="psA", bufs=3, space="PSUM"))
            psL = pctx.enter_context(tc.tile_pool(name="psL", bufs=1, space="PSUM"))
            psT = pctx.enter_context(tc.tile_pool(name="psT", bufs=1, space="PSUM"))

            xpcs = {}

            def load_xpc(c):
                t = xpool.tile([128, CHUNK], BF16, name="xpc", tag="xpc")
                nc.sync.dma_start(t[:], xpr_d[c])
                xpcs[c] = t

            def do_exp(c, lg):
                # exp of chunk c's logits (transposes are deferred and
                # interleaved into the next chunk's sh rounds)
                e_c = epool.tile([128, CHUNK], BF16, name="e_c", tag="e_c")
                nc.scalar.activation(e_c[:], lg[:], AF.Exp)
                return e_c

            def do_tp(c, e_c, j):
                t = c * (CHUNK // 128) + j
                t_ps = psT.tile([128, 128], BF16, name="t_ps", tag="tp")
                nc.tensor.transpose(
                    t_ps[:], e_c[:, j * 128 : (j + 1) * 128], id128_s[:]
                )
                nc.vector.tensor_copy(
                    out=enm[:, t * 128 : (t + 1) * 128], in_=t_ps[:]
                )

            def expose(c, lg):
                e_c = do_exp(c, lg)
                for j in range(CHUNK // 128):
                    do_tp(c, e_c, j)

            def produce_sh(c, prev=None, prev2=None):
                # interleave: after issuing round r of chunk c's sh matmuls,
                # issue round r of chunk c-1's lg matmuls (inputs all ready)
                # and, every other round, a deferred transpose of chunk c-2,
                # so all PE work rides inside the evac-paced sh stream.
                if prev is not None:
                    pc, pshts = prev
                    plg = psL.tile([128, CHUNK], f32, name="lg_ps", tag="lg")
                xpc = xpcs.pop(c)
                shts = []
                for r in range(8):
                    if prev is not None:
                        for i in range(4):
                            h0 = (i % 2) * CHUNK
                            nc.tensor.matmul(
                                plg[32 * i : 32 * i + 32, :],
                                w2r_s[:, r * 32 : (r + 1) * 32],
                                pshts[r * 2 + i // 2][:, h0 : h0 + CHUNK],
                                start=(r == 0),
                                stop=(r == 7),
                                skip_group_check=True,
                                tile_position=(0, 32 * i),
                            )
                    if prev2 is not None and r % 2 == 1:
                        do_tp(prev2[0], prev2[1], r // 2)
                    # 8 PE tiles in 32x64 mode: tile (i,j2) computes slots
                    # 2*j2,2*j2+1 (64 dims) for group i; diagonal issue order
                    # keeps consecutive matmuls in distinct row-groups.
                    # Groups are paired into lazily-allocated 2-bank psum
                    # tiles (group 2p at cols 0:512, 2p+1 at 512:1024) so a
                    # single relu-evac instruction covers two groups.
                    pairs = [None] * 2
                    for q in range(2):
                        for i in range(4):
                            j2 = (i + q) % 2
                            if pairs[i // 2] is None:
                                pairs[i // 2] = psA.tile(
                                    [128, 2 * CHUNK], f32, name="sh_ps", tag="shp"
                                )
                            h0 = (i % 2) * CHUNK
                            nc.tensor.matmul(
                                pairs[i // 2][
                                    64 * j2 : 64 * j2 + 64, h0 : h0 + CHUNK
                                ],
                                wbd_s[
                                    32 * i : 32 * i + 18,
                                    r * 128 + 64 * j2 : r * 128 + 64 * j2 + 64,
                                ],
                                xpc[32 * i : 32 * i + 18, :],
                                start=True,
                                stop=True,
                                skip_group_check=True,
                                tile_position=(32 * i, 64 * j2),
                            )
                    for p in range(2):
                        st = shpool.tile(
                            [128, 2 * CHUNK], BF16, name="sh_sb", tag="shs"
                        )
                        if (r * 2 + p) % 2 == 0:
                            nc.scalar.activation(st[:], pairs[p][:], AF.Relu)
                        else:
                            nc.vector.tensor_scalar(
                                st[:], pairs[p][:], 0.0, None, ALU.max
                            )
                        shts.append(st)
                if prev is not None:
                    e_c = do_exp(pc, plg)
                    return shts, (pc, e_c)
                return shts, None

            def do_lg(c, shts):
                lg_ps = psL.tile([128, CHUNK], f32, name="lg_ps", tag="lg")
                for r in range(8):
                    for i in range(4):
                        h0 = (i % 2) * CHUNK
                        nc.tensor.matmul(
                            lg_ps[32 * i : 32 * i + 32, :],
                            w2r_s[:, r * 32 : (r + 1) * 32],
                            shts[r * 2 + i // 2][:, h0 : h0 + CHUNK],
                            start=(r == 0),
                            stop=(r == 7),
                            skip_group_check=True,
                            tile_position=(0, 32 * i),
                        )
                expose(c, lg_ps)

            # software pipeline: sh(c) production overlaps lg/expose(c-1), so
            # the relu evacuators (DVE+ACT) never idle behind the PE tail.
            load_xpc(0)
            if NCH > 1:
                load_xpc(1)
            prev = None
            prev2 = None
            for c in range(NCH):
                if c + 2 < NCH:
                    load_xpc(c + 2)
                shts, expd = produce_sh(c, prev, prev2)
                prev = (c, shts)
                prev2 = expd
            # drain the pipeline: lg+exp of the last chunk, remaining
            # transposes of the last two chunks
            if prev2 is not None:
                for j in range(CHUNK // 128):
                    do_tp(prev2[0], prev2[1], j)
            do_lg(prev[0], prev[1])

    if "P" in phases:
        # ---- Phase C2: n-major encoder hidden + weighted pooling ---------
        with ExitStack() as pctx:
            xbpool = pctx.enter_context(tc.tile_pool(name="xpC", bufs=2))
            h1pool = pctx.enter_context(tc.tile_pool(name="h1p", bufs=3))
            smpool = pctx.enter_context(tc.tile_pool(name="smp", bufs=2))
            psH = pctx.enter_context(tc.tile_pool(name="psH", bufs=4, space="PSUM"))
            psC = pctx.enter_context(tc.tile_pool(name="psC", bufs=2, space="PSUM"))
            psU = pctx.enter_context(tc.tile_pool(name="psU", bufs=1, space="PSUM"))
            TB = 4  # blocks per psum batch
            xpbs = {}

            def load_xpb(b):
                t = xbpool.tile([6, N], BF16, name="xpb", tag="xpb")
                nc.sync.dma_start(t[:], xp_d[b])
                xpbs[b] = t

            load_xpb(0)
            for b in range(BPC):
                if b + 1 < BPC:
                    load_xpb(b + 1)
                xpb = xpbs.pop(b)
                c1_ps = psC.tile([128, 65], f32, name="c1_ps", tag="c1")
                hpend = []

                def drain_pool(c1_ps=c1_ps, b=b):
                    # col-tile jj = t%4: 4 pooling matmuls run concurrently in
                    # 4 PE column-tiles, accumulating partials at partition
                    # offsets 32*jj; partials are summed after the block loop.
                    # lhsT is the full 32-col enm group of b (cols for other
                    # b' in the group give junk-but-finite rows the selector
                    # matmul ignores; this keeps every partition written).
                    tg, h1_sb = hpend.pop(0)
                    g = 32 * (b % 4)
                    for j in range(TB):
                        t = tg * TB + j
                        jj = t % 4
                        nc.tensor.matmul(
                            c1_ps[32 * jj : 32 * jj + 32, :],
                            enm[:, t * 128 + g : t * 128 + g + 32],
                            h1_sb[:, j * 65 : (j + 1) * 65],
                            start=(t < 4),
                            stop=(t >= NB - 4),
                            skip_group_check=True,
                            tile_position=(0, 32 * jj),
                        )

                for tg in range(NB // TB):
                    h1_ps = psH.tile([128, TB * 65], f32, name="h1_ps", tag="h1")
                    for j in range(TB):
                        t = tg * TB + j
                        nc.tensor.matmul(
                            h1_ps[:, j * 65 : (j + 1) * 65],
                            xpb[:, t * 128 : (t + 1) * 128],
                            wenm_s[:, b * 65 : (b + 1) * 65],
                            start=True,
                            stop=True,
                            skip_group_check=True,
                        )
                    h1_sb = h1pool.tile([128, TB * 65], BF16, name="h1_sb", tag="h1s")
                    nc.vector.tensor_scalar(
                        h1_sb[:], h1_ps[:], 0.0, None, ALU.max
                    )
                    hpend.append((tg, h1_sb))
                    if len(hpend) > 1:
                        drain_pool()
                while hpend:
                    drain_pool()
                # sum the 4 col-tile partials (partition offsets 0/32/64/96)
                # via a 0/1-selector matmul: csum[h] = sum_jj c1[32jj+h]
                c1f = smpool.tile([128, 65], f32, name="c1f", tag="c1f")
                nc.vector.tensor_copy(out=c1f[:], in_=c1_ps[:])
                csum_ps = psU.tile([4, 65], f32, name="csum_ps", tag="csum")
                bi4 = 4 * (b // 4)
                nc.tensor.matmul(
                    csum_ps[:], sel4_s[:, bi4 : bi4 + 4], c1f[:],
                    start=True, stop=True,
                )
                rz = smpool.tile([4, 1], f32, name="rz", tag="rz")
                nc.vector.reciprocal(rz[:], csum_ps[:, 64:65])
                ctxn = smpool.tile([4, 64], f32, name="ctxn", tag="ctxn")
                nc.vector.tensor_scalar_mul(ctxn[:], csum_ps[:, 0:64], rz[:])
                tp_ps = psU.tile([64, 4], f32, name="tp_ps", tag="tp2")
                nc.tensor.transpose(tp_ps[:], ctxn[:], id64f_s[0:4, 0:4])
                nc.vector.tensor_copy(
                    out=ctxnT[0:64, b * 4 : (b + 1) * 4], in_=tp_ps[:]
                )

    if "D" in phases:
        # ---- Phase D: pooled-context encoder layer 2 + output MLP --------
        with ExitStack() as pctx:
            psD = pctx.enter_context(tc.tile_pool(name="psD", bufs=1, space="PSUM"))
            fct_ps = psD.tile([64, 64], f32, name="fct_ps", tag="fctp")
            nc.tensor.matmul(fct_ps[:], ew2b_s[:], ctxnT[:], start=True, stop=True)
            nc.vector.tensor_copy(out=fct[:], in_=fct_ps[:])
            fct_bh = fct.rearrange("d (b h) -> d b h", h=HEADS)
            o1_ps = psD.tile([64, BPC], f32, name="o1_ps", tag="o1p")
            for h in range(HEADS):
                nc.tensor.matmul(
                    o1_ps[:],
                    ow1_s[:, h * 64 : (h + 1) * 64],
                    fct_bh[:, :, h],
                    start=(h == 0),
                    stop=False,
                    skip_group_check=True,
                )
            nc.tensor.matmul(
                o1_ps[:], ob1_s[:], on16_s[:], start=False, stop=True,
                skip_group_check=True,
            )
            nc.scalar.activation(obuf[0:64, :], o1_ps[:], AF.Relu)
            fin_ps = psD.tile([1, BPC], f32, name="fin_ps", tag="finp")
            nc.tensor.matmul(fin_ps[:], w2o_s[:], obuf[:], start=True, stop=True)
            nc.vector.tensor_copy(out=res[:], in_=fin_ps[:])
            nc.sync.dma_start(out_d.rearrange("(a n) -> a n", a=1), res[:])


def make_in_maps(inputs):
    """Host-side marshalling: slice B across cores and pack weights into the
    layouts the device program expects.

    bf16 note: the big streamed matmuls run in bf16.  To avoid systematic
    model-weight rounding, layer-1 weights are split hi/lo across extra
    contraction rows (w = hi + lo with both bf16); per-point input rounding
    is stochastic and averages out in the softmax pooling."""
    import ml_dtypes

    bf = ml_dtypes.bfloat16
    f = np.float32

    def split(a):
        hi = a.astype(bf)
        lo = (a - hi.astype(f)).astype(bf)
        return hi, lo
    radar = np.concatenate(
        [np.asarray(inputs["radar_xy"], f), np.asarray(inputs["radar_dir"], f)], axis=1
    )  # [B, 4]
    pts = np.asarray(inputs["pts"], f)
    enc_w1 = np.asarray(inputs["enc_w1"], f)
    enc_b1 = np.asarray(inputs["enc_b1"], f)
    enc_w2 = np.asarray(inputs["enc_w2"], f)
    enc_b2 = np.asarray(inputs["enc_b2"], f)
    sc_w1 = np.asarray(inputs["sc_w1"], f)
    sc_b1 = np.asarray(inputs["sc_b1"], f)
    sc_w2 = np.asarray(inputs["sc_w2"], f)
    out_w1 = np.asarray(inputs["out_w1"], f)
    out_b1 = np.asarray(inputs["out_b1"], f)
    out_w2 = np.asarray(inputs["out_w2"], f)
    out_b2 = np.asarray(inputs["out_b2"], f)

    # per-b layer-1 bias vectors (radar is constant over n)
    cb_sc = np.einsum("br,hrd->bhd", radar, sc_w1[:, :4, :]) + sc_b1  # [B, 4, 64]
    cb_enc = radar @ enc_w1[:4] + enc_b1  # [B, 64]

    # xp rows: [xh, yh, xh, yh, 1, 1] (bf16); rows 0-3 feed the weight-split
    # layer-1 matmuls, rows 4-5 carry the (split) bias contraction.
    xp = np.empty((B, 6, N), bf)
    xh = pts[:, :, 0].astype(bf)
    yh = pts[:, :, 1].astype(bf)
    xp[:, 0] = xh
    xp[:, 1] = yh
    xp[:, 2] = xh
    xp[:, 3] = yh
    xp[:, 4] = 1.0
    xp[:, 5] = 1.0

    # Round r of the score layer covers half s=r//4 (heads 2s,2s+1), hidden
    # dims D = 32*(r%4)+d; head hd(r,d) = 2s + D//64, within-head dim D%64.
    def rhd(r, d):
        D = 32 * (r % 4) + d
        return 2 * (r // 4) + D // 64, D % 64

    # w2r: lg lhsT per round: row 32*sl+d, col 4*sl+hd -> logit (slot sl,
    # head hd); cols 16-31 stay zero (pad to a 32-wide col-tile).
    w2r = np.zeros((128, 8 * 32), bf)
    for r in range(8):
        for sl in range(4):
            for d in range(32):
                hd, dm = rhd(r, d)
                w2r[32 * sl + d, r * 32 + 4 * sl + hd] = sc_w2[hd, dm]
    # pooling partial-sum selector: col 4*bi+h sums rows {32*jj+4*bi+h}
    sel4 = np.zeros((128, 16), f)
    for bi in range(4):
        for h in range(4):
            for jj in range(4):
                sel4[32 * jj + 4 * bi + h, 4 * bi + h] = 1.0

    ew2b = np.concatenate([enc_w2, enc_b2[None, :]], axis=0)  # [65, 64]
    ow1 = np.empty((64, 256), f)
    for h in range(HEADS):
        ow1[:, h * 64 : (h + 1) * 64] = out_w1[h * 64 : (h + 1) * 64, :]
    ob1 = np.ascontiguousarray(out_b1[None, :])
    w2o = np.concatenate([out_w2, out_b2[None, :]], axis=0)  # [65, 1]
    id128 = np.eye(128, dtype=bf)
    id64f = np.eye(64, dtype=f)
    on16 = np.ones((1, BPC), f)

    # layer-1 weight splits for wbd
    wxh_s, wxl_s = split(sc_w1[:, 4, :])  # [4, 64] each
    wyh_s, wyl_s = split(sc_w1[:, 5, :])
    cbh_s, cbl_s = split(cb_sc)  # [B, 4, 64]

    in_maps = []
    for c in range(NCORES):
        sl = slice(c * BPC, (c + 1) * BPC)
        # wenm rows [wxh, wyh, wxl, wyl, bh, bl] vs xp rows [xh, yh, xh, yh, 1, 1]
        wenm = np.zeros((6, BPC * 65), bf)
        exh, exl = split(enc_w1[4])
        eyh, eyl = split(enc_w1[5])
        for bl in range(BPC):
            s = slice(bl * 65, bl * 65 + 64)
            wenm[0, s] = exh
            wenm[1, s] = eyh
            wenm[2, s] = exl
            wenm[3, s] = eyl
            bh, blo = split(cb_enc[c * BPC + bl])
            wenm[4, s] = bh
            wenm[5, s] = blo
            wenm[4, bl * 65 + 64] = 1.0
        xpc_core = np.ascontiguousarray(xp[sl])
        # wbd: block-diag score layer-1 lhsT per (round r, group g): rows
        # 32g+4sl+[xh,yh,xl,yl weights], rows 32g+16/17 = hi/lo bias (b=4sl+g)
        wbd = np.zeros((128, 8 * 128), bf)
        for r in range(8):
            hds = np.empty(128, np.int64)
            dms = np.empty(128, np.int64)
            for slt in range(4):
                for d in range(32):
                    hd, dm = rhd(r, d)
                    hds[32 * slt + d] = hd
                    dms[32 * slt + d] = dm
            for g in range(4):
                for slt in range(4):
                    cols = np.arange(r * 128 + 32 * slt, r * 128 + 32 * slt + 32)
                    h_v = hds[32 * slt : 32 * slt + 32]
                    d_v = dms[32 * slt : 32 * slt + 32]
                    wbd[32 * g + 4 * slt + 0, cols] = wxh_s[h_v, d_v]
                    wbd[32 * g + 4 * slt + 1, cols] = wyh_s[h_v, d_v]
                    wbd[32 * g + 4 * slt + 2, cols] = wxl_s[h_v, d_v]
                    wbd[32 * g + 4 * slt + 3, cols] = wyl_s[h_v, d_v]
                    gb = c * BPC + 4 * slt + g
                    wbd[32 * g + 16, cols] = cbh_s[gb, h_v, d_v]
                    wbd[32 * g + 17, cols] = cbl_s[gb, h_v, d_v]
        # xpr: per chunk [128, CHUNK]: rows 32g+4sl+[xh,yh,xh,yh] of b=4sl+g,
        # rows 32g+16/17 = ones (bias contraction)
        xpr = np.zeros((NCH, 128, CHUNK), bf)
        for lb in range(BPC):
            g, slt = lb % 4, lb // 4
            gb = c * BPC + lb
            xb = xh[gb].reshape(NCH, CHUNK)
            yb = yh[gb].reshape(NCH, CHUNK)
            xpr[:, 32 * g + 4 * slt + 0, :] = xb
            xpr[:, 32 * g + 4 * slt + 1, :] = yb
            xpr[:, 32 * g + 4 * slt + 2, :] = xb
            xpr[:, 32 * g + 4 * slt + 3, :] = yb
        for g in range(4):
            xpr[:, 32 * g + 16, :] = 1.0
            xpr[:, 32 * g + 17, :] = 1.0
        in_maps.append(
            dict(
                xp=xpc_core,
                xpr=xpr,
                wbd=wbd,
                w2r=w2r,
                wenm=wenm,
                ew2b=ew2b,
                ow1=ow1,
                ob1=ob1,
                w2o=w2o,
                id128=id128,
                id64f=id64f,
                sel4=sel4,
                on16=on16,
            )
        )
    return in_maps


_CACHE = {}


def _get_runner():
    """Build the Bass program once and a cached jitted PJRT executable over
    the 8 cores (shard_map along axis 0 of every input)."""
    if "runner" in _CACHE:
        return _CACHE["runner"]

    import jax
    from jax.sharding import Mesh, NamedSharding, PartitionSpec

    from concourse.bass2jax import (
        _bass_exec_p,
        install_neuronx_cc_hook,
        partition_id_tensor,
        shard_map,
    )

    nc = build_nc()
    install_neuronx_cc_hook()
    partition_name = nc.partition_id_tensor.name if nc.partition_id_tensor else None
    in_names, out_names, out_avals = [], [], []
    for alloc in nc.m.functions[0].allocations:
        if not isinstance(alloc, mybir.MemoryLocationSet):
            continue
        name = alloc.memorylocations[0].name
        if alloc.kind == "ExternalInput":
            if name != partition_name:
                in_names.append(name)
        elif alloc.kind == "ExternalOutput":
            out_names.append(name)
            out_avals.append(
                jax.core.ShapedArray(tuple(alloc.tensor_shape), mybir.dt.np(alloc.dtype))
            )
    all_in_names = tuple(in_names + out_names)
    if partition_name is not None:
        all_in_names = all_in_names + (partition_name,)

    def _body(*args):
        operands = list(args)
        if partition_name is not None:
            operands.append(partition_id_tensor())
        return tuple(
            _bass_exec_p.bind(
                *operands,
                out_avals=tuple(out_avals),
                in_names=all_in_names,
                out_names=tuple(out_names),
                lowering_input_output_aliases=(),
                sim_require_finite=True,
                sim_require_nnan=True,
                nc=nc,
            )
        )

    devices = jax.devices()[:NCORES]
    mesh = Mesh(np.asarray(devices), ("core",))
    nin = len(in_names) + len(out_names)
    fn = jax.jit(
        shard_map(
            _body,
            mesh=mesh,
            in_specs=(PartitionSpec("core"),) * nin,
            out_specs=(PartitionSpec("core"),) * len(out_names),
            check_rep=False,
        ),
        keep_unused=True,
    )
    sharding = NamedSharding(mesh, PartitionSpec("core"))
    runner = (fn, sharding, in_names, out_avals)
    _CACHE["runner"] = runner
    return runner


def kernel(**inputs):
    import jax

    in_maps = make_in_maps(inputs)
    fn, sharding, in_names, out_avals = _get_runner()
    concat_in = [
        np.concatenate([np.asarray(in_maps[c][name]) for c in range(NCORES)], axis=0)
        for name in in_names
    ]
    concat_zeros = [
        np.zeros((NCORES * a.shape[0], *a.shape[1:]), a.dtype) for a in out_avals
    ]
    args = [jax.device_put(a, sharding) for a in (*concat_in, *concat_zeros)]
    (out,) = fn(*args)
    return np.asarray(out).reshape(B).astype(np.float32)



# revision 77
# speedup vs baseline: 1.1371x; 1.1371x over previous
"""Trainium2 Bass kernel for nn_MultiHeadMLPAttentionModel.

Model: per (b, n) point: pairwise = [radar_b(4), pt(2)] (radar constant over n).
  h1 = relu(pairwise @ enc_w1 + enc_b1)            [B,N,64]
  pf = h1 @ enc_w2 + enc_b2                        [B,N,64]
  sh = relu(einsum('bnf,hfd', pairwise, sc_w1) + sc_b1)
  logits = einsum('bnhd,hd', sh, sc_w2) + sc_b2    [B,N,4]
  w = softmax(logits, axis=n)
  ctx = einsum('bnh,bnd', w, pf)  -> out MLP -> [B]

Key algebraic restructurings used here:
  * pooling commutes with the (linear) second encoder layer since softmax
    weights sum to 1:  ctx = (sum_n w * h1) @ enc_w2 + enc_b2.  This removes
    the N-scale enc2 matmul entirely.
  * sc_b2 is constant over n, so it drops out of the softmax.
  * the radar part of pairwise is constant over n, so all layer-1 radar
    contributions fold into per-b bias vectors (computed on host: ~200 KFLOP
    of the model's 13 GFLOP).
  * softmax is computed without max-subtraction (logits are O(1) for this
    model; exp is evaluated in fp32) and normalization is deferred: the
    pooling matmul accumulates unnormalized sum_n exp(l)*h1 plus sum_n exp(l)
    (via an appended ones column), and the division happens once per b.

Sharding: pure data parallel over B: 8 cores x 16 rows each.  One SPMD Bass
program; per-core inputs differ only in data.
"""

import numpy as np

import concourse.bass as bass
import concourse.tile as tile
from concourse import bacc, mybir

B, N, HID, HEADS = 128, 8192, 64, 4
NCORES = 8
BPC = B // NCORES  # 16 batch rows per core
CHUNK = 512
NCH = N // CHUNK  # 16
NB = N // 128  # 64 point-blocks of 128

F32 = mybir.dt.float32
BF16 = mybir.dt.bfloat16
AF = mybir.ActivationFunctionType
ALU = mybir.AluOpType


def build_nc(reps=1, phases="ATPD"):
    from contextlib import ExitStack

    nc = bacc.Bacc()
    f32 = F32

    xp_d = nc.dram_tensor("xp", [BPC, 6, N], BF16, kind="ExternalInput")
    xpr_d = nc.dram_tensor("xpr", [NCH, 128, CHUNK], BF16, kind="ExternalInput")
    wbd_d = nc.dram_tensor("wbd", [128, 8 * 128], BF16, kind="ExternalInput")
    w2r_d = nc.dram_tensor("w2r", [128, 8 * 32], BF16, kind="ExternalInput")
    wenm_d = nc.dram_tensor("wenm", [6, BPC * 65], BF16, kind="ExternalInput")
    ew2b_d = nc.dram_tensor("ew2b", [65, 64], f32, kind="ExternalInput")
    ow1_d = nc.dram_tensor("ow1", [64, 256], f32, kind="ExternalInput")
    ob1_d = nc.dram_tensor("ob1", [1, 64], f32, kind="ExternalInput")
    w2o_d = nc.dram_tensor("w2o", [65, 1], f32, kind="ExternalInput")
    id128_d = nc.dram_tensor("id128", [128, 128], BF16, kind="ExternalInput")
    id64f_d = nc.dram_tensor("id64f", [64, 64], f32, kind="ExternalInput")
    sel4_d = nc.dram_tensor("sel4", [128, 16], f32, kind="ExternalInput")
    on16_d = nc.dram_tensor("on16", [1, BPC], f32, kind="ExternalInput")
    out_d = nc.dram_tensor("out", [BPC], f32, kind="ExternalOutput")

    with tile.TileContext(nc) as tc, ExitStack() as ctx:
        consts = ctx.enter_context(tc.tile_pool(name="consts", bufs=1))

        def cload(dram, shape, nm, dt=f32):
            t = consts.tile(shape, dt, name=nm, tag=nm)
            nc.sync.dma_start(t[:], dram[:])
            return t

        wbd_s = cload(wbd_d, [128, 8 * 128], "wbd_s", BF16)
        w2r_s = cload(w2r_d, [128, 8 * 32], "w2r_s", BF16)
        wenm_s = cload(wenm_d, [6, BPC * 65], "wenm_s", BF16)
        ew2b_s = cload(ew2b_d, [65, 64], "ew2b_s")
        ow1_s = cload(ow1_d, [64, 256], "ow1_s")
        ob1_s = cload(ob1_d, [1, 64], "ob1_s")
        w2o_s = cload(w2o_d, [65, 1], "w2o_s")
        id128_s = cload(id128_d, [128, 128], "id128_s", BF16)
        id64f_s = cload(id64f_d, [64, 64], "id64f_s")
        sel4_s = cload(sel4_d, [128, 16], "sel4_s")
        on16_s = cload(on16_d, [1, BPC], "on16_s")

        # n-major exp(logits): block t occupies cols [t*128, (t+1)*128), within
        # a block: partition p = n offset, col = 32*(b%4) + 4*(b//4) + h
        enm = consts.tile([128, NB * 128], BF16, name="enm", tag="enm")
        ctxnT = consts.tile([65, 64], f32, name="ctxnT", tag="ctxnT")
        obuf = consts.tile([65, BPC], f32, name="obuf", tag="obuf")
        fct = consts.tile([64, 64], f32, name="fct", tag="fct")
        res = consts.tile([1, BPC], f32, name="res", tag="res")
        nc.vector.memset(ctxnT[64:65, :], 1.0)
        nc.vector.memset(obuf[64:65, :], 1.0)

        if "A" not in phases:
            nc.vector.memset(enm[:, 0:8], 0.0)
        for _rep in range(reps):
            _build_body(
                nc, tc, xp_d, xpr_d, out_d,
                wbd_s, w2r_s, wenm_s, ew2b_s, ow1_s,
                ob1_s, w2o_s, id128_s, id64f_s, sel4_s, on16_s,
                enm, ctxnT, obuf, fct, res, phases,
            )

    if not nc.is_finalized():
        nc.finalize()
    return nc


def _build_body(
    nc, tc, xp_d, xpr_d, out_d,
    wbd_s, w2r_s, wenm_s, ew2b_s, ow1_s,
    ob1_s, w2o_s, id128_s, id64f_s, sel4_s, on16_s,
    enm, ctxnT, obuf, fct, res, phases="ATPD",
):
    from contextlib import ExitStack

    f32 = F32
    if "A" in phases:
        # ---- Phase A: score-net hidden + logits ---------------------------
        # sh: block-diagonal packing, 4 b's per matmul (K=18: 16 data rows +
        # 2 bias ones-rows), 4 concurrent row-tiles (one per b%4 group).
        # Round r covers 32 hidden dims; psum tile (r,i) = [4 slots x 32
        # dims, 512 pts].  Evac = pure relu (bias folded into the matmul).
        # lg: per (r,i) one K=128 matmul contracts the whole evac'd tile
        # (block-diag w2 slice), 4 concurrent col-tiles, accumulating over r.
        with ExitStack() as pctx:
            xpool = pctx.enter_context(tc.tile_pool(name="xpA", bufs=3))
            shpool = pctx.enter_context(tc.tile_pool(name="shp", bufs=68))
            epool = pctx.enter_context(tc.tile_pool(name="ep", bufs=2))
            psA = pctx.enter_context(tc.tile_pool(name="psA", bufs=6, space="PSUM"))
            psL = pctx.enter_context(tc.tile_pool(name="psL", bufs=1, space="PSUM"))
            psT = pctx.enter_context(tc.tile_pool(name="psT", bufs=1, space="PSUM"))

            xpcs = {}

            def load_xpc(c):
                t = xpool.tile([128, CHUNK], BF16, name="xpc", tag="xpc")
                nc.sync.dma_start(t[:], xpr_d[c])
                xpcs[c] = t

            def do_exp(c, lg):
                # exp of chunk c's logits (transposes are deferred and
                # interleaved into the next chunk's sh rounds)
                e_c = epool.tile([128, CHUNK], BF16, name="e_c", tag="e_c")
                nc.scalar.activation(e_c[:], lg[:], AF.Exp)
                return e_c

            def do_tp(c, e_c, j):
                t = c * (CHUNK // 128) + j
                t_ps = psT.tile([128, 128], BF16, name="t_ps", tag="tp")
                nc.tensor.transpose(
                    t_ps[:], e_c[:, j * 128 : (j + 1) * 128], id128_s[:]
                )
                nc.vector.tensor_copy(
                    out=enm[:, t * 128 : (t + 1) * 128], in_=t_ps[:]
                )

            def expose(c, lg):
                e_c = do_exp(c, lg)
                for j in range(CHUNK // 128):
                    do_tp(c, e_c, j)

            def produce_sh(c, prev=None, prev2=None):
                # interleave: after issuing round r of chunk c's sh matmuls,
                # issue round r of chunk c-1's lg matmuls (inputs all ready)
                # and, every other round, a deferred transpose of chunk c-2,
                # so all PE work rides inside the evac-paced sh stream.
                if prev is not None:
                    pc, pshts = prev
                    plg = psL.tile([128, CHUNK], f32, name="lg_ps", tag="lg")
                xpc = xpcs.pop(c)
                shts = []
                for r in range(8):
                    if prev is not None:
                        for i in range(4):
                            nc.tensor.matmul(
                                plg[32 * i : 32 * i + 32, :],
                                w2r_s[:, r * 32 : (r + 1) * 32],
                                pshts[r * 4 + i][:],
                                start=(r == 0),
                                stop=(r == 7),
                                skip_group_check=True,
                                tile_position=(0, 32 * i),
                            )
                    if prev2 is not None and r % 2 == 1:
                        do_tp(prev2[0], prev2[1], r // 2)
                    # 8 PE tiles in 32x64 mode: tile (i,j2) computes slots
                    # 2*j2,2*j2+1 (64 dims) for group i; diagonal issue order
                    # keeps consecutive matmuls in distinct row-groups and
                    # halves the per-bank writer count vs 32x32 tiling.
                    # psum tiles are allocated lazily per group in the q=0
                    # wave so production streams as banks free one-by-one.
                    pss = [None] * 4
                    for q in range(2):
                        for i in range(4):
                            j2 = (i + q) % 2
                            if pss[i] is None:
                                pss[i] = psA.tile(
                                    [128, CHUNK], f32, name="sh_ps", tag="shp"
                                )
                            nc.tensor.matmul(
                                pss[i][64 * j2 : 64 * j2 + 64, :],
                                wbd_s[
                                    32 * i : 32 * i + 18,
                                    r * 128 + 64 * j2 : r * 128 + 64 * j2 + 64,
                                ],
                                xpc[32 * i : 32 * i + 18, :],
                                start=True,
                                stop=True,
                                skip_group_check=True,
                                tile_position=(32 * i, 64 * j2),
                            )
                    for i in range(4):
                        st = shpool.tile([128, CHUNK], BF16, name="sh_sb", tag="shs")
                        if (r * 4 + i) % 2 == 0:
                            nc.scalar.activation(st[:], pss[i][:], AF.Relu)
                        else:
                            nc.vector.tensor_scalar(
                                st[:], pss[i][:], 0.0, None, ALU.max
                            )
                        shts.append(st)
                if prev is not None:
                    e_c = do_exp(pc, plg)
                    return shts, (pc, e_c)
                return shts, None

            def do_lg(c, shts):
                lg_ps = psL.tile([128, CHUNK], f32, name="lg_ps", tag="lg")
                for r in range(8):
                    for i in range(4):
                        nc.tensor.matmul(
                            lg_ps[32 * i : 32 * i + 32, :],
                            w2r_s[:, r * 32 : (r + 1) * 32],
                            shts[r * 4 + i][:],
                            start=(r == 0),
                            stop=(r == 7),
                            skip_group_check=True,
                            tile_position=(0, 32 * i),
                        )
                expose(c, lg_ps)

            # software pipeline: sh(c) production overlaps lg/expose(c-1), so
            # the relu evacuators (DVE+ACT) never idle behind the PE tail.
            load_xpc(0)
            if NCH > 1:
                load_xpc(1)
            prev = None
            prev2 = None
            for c in range(NCH):
                if c + 2 < NCH:
                    load_xpc(c + 2)
                shts, expd = produce_sh(c, prev, prev2)
                prev = (c, shts)
                prev2 = expd
            # drain the pipeline: lg+exp of the last chunk, remaining
            # transposes of the last two chunks
            if prev2 is not None:
                for j in range(CHUNK // 128):
                    do_tp(prev2[0], prev2[1], j)
            do_lg(prev[0], prev[1])

    if "P" in phases:
        # ---- Phase C2: n-major encoder hidden + weighted pooling ---------
        with ExitStack() as pctx:
            xbpool = pctx.enter_context(tc.tile_pool(name="xpC", bufs=2))
            h1pool = pctx.enter_context(tc.tile_pool(name="h1p", bufs=3))
            smpool = pctx.enter_context(tc.tile_pool(name="smp", bufs=2))
            psH = pctx.enter_context(tc.tile_pool(name="psH", bufs=4, space="PSUM"))
            psC = pctx.enter_context(tc.tile_pool(name="psC", bufs=2, space="PSUM"))
            psU = pctx.enter_context(tc.tile_pool(name="psU", bufs=1, space="PSUM"))
            TB = 4  # blocks per psum batch
            xpbs = {}

            def load_xpb(b):
                t = xbpool.tile([6, N], BF16, name="xpb", tag="xpb")
                nc.sync.dma_start(t[:], xp_d[b])
                xpbs[b] = t

            load_xpb(0)
            for b in range(BPC):
                if b + 1 < BPC:
                    load_xpb(b + 1)
                xpb = xpbs.pop(b)
                c1_ps = psC.tile([128, 65], f32, name="c1_ps", tag="c1")
                hpend = []

                def drain_pool(c1_ps=c1_ps, b=b):
                    # col-tile jj = t%4: 4 pooling matmuls run concurrently in
                    # 4 PE column-tiles, accumulating partials at partition
                    # offsets 32*jj; partials are summed after the block loop.
                    # lhsT is the full 32-col enm group of b (cols for other
                    # b' in the group give junk-but-finite rows the selector
                    # matmul ignores; this keeps every partition written).
                    tg, h1_sb = hpend.pop(0)
                    g = 32 * (b % 4)
                    for j in range(TB):
                        t = tg * TB + j
                        jj = t % 4
                        nc.tensor.matmul(
                            c1_ps[32 * jj : 32 * jj + 32, :],
                            enm[:, t * 128 + g : t * 128 + g + 32],
                            h1_sb[:, j * 65 : (j + 1) * 65],
                            start=(t < 4),
                            stop=(t >= NB - 4),
                            skip_group_check=True,
                            tile_position=(0, 32 * jj),
                        )

                for tg in range(NB // TB):
                    h1_ps = psH.tile([128, TB * 65], f32, name="h1_ps", tag="h1")
                    for j in range(TB):
                        t = tg * TB + j
                        nc.tensor.matmul(
                            h1_ps[:, j * 65 : (j + 1) * 65],
                            xpb[:, t * 128 : (t + 1) * 128],
                            wenm_s[:, b * 65 : (b + 1) * 65],
                            start=True,
                            stop=True,
                            skip_group_check=True,
                        )
                    h1_sb = h1pool.tile([128, TB * 65], BF16, name="h1_sb", tag="h1s")
                    nc.vector.tensor_scalar(
                        h1_sb[:], h1_ps[:], 0.0, None, ALU.max
                    )
                    hpend.append((tg, h1_sb))
                    if len(hpend) > 1:
                        drain_pool()
                while hpend:
                    drain_pool()
                # sum the 4 col-tile partials (partition offsets 0/32/64/96)
                # via a 0/1-selector matmul: csum[h] = sum_jj c1[32jj+h]
                c1f = smpool.tile([128, 65], f32, name="c1f", tag="c1f")
                nc.vector.tensor_copy(out=c1f[:], in_=c1_ps[:])
                csum_ps = psU.tile([4, 65], f32, name="csum_ps", tag="csum")
                bi4 = 4 * (b // 4)
                nc.tensor.matmul(
                    csum_ps[:], sel4_s[:, bi4 : bi4 + 4], c1f[:],
                    start=True, stop=True,
                )
                rz = smpool.tile([4, 1], f32, name="rz", tag="rz")
                nc.vector.reciprocal(rz[:], csum_ps[:, 64:65])
                ctxn = smpool.tile([4, 64], f32, name="ctxn", tag="ctxn")
                nc.vector.tensor_scalar_mul(ctxn[:], csum_ps[:, 0:64], rz[:])
                tp_ps = psU.tile([64, 4], f32, name="tp_ps", tag="tp2")
                nc.tensor.transpose(tp_ps[:], ctxn[:], id64f_s[0:4, 0:4])
                nc.vector.tensor_copy(
                    out=ctxnT[0:64, b * 4 : (b + 1) * 4], in_=tp_ps[:]
                )

    if "D" in phases:
        # ---- Phase D: pooled-context encoder layer 2 + output MLP --------
        with ExitStack() as pctx:
            psD = pctx.enter_context(tc.tile_pool(name="psD", bufs=1, space="PSUM"))
            fct_ps = psD.tile([64, 64], f32, name="fct_ps", tag="fctp")
            nc.tensor.matmul(fct_ps[:], ew2b_s[:], ctxnT[:], start=True, stop=True)
            nc.vector.tensor_copy(out=fct[:], in_=fct_ps[:])
            fct_bh = fct.rearrange("d (b h) -> d b h", h=HEADS)
            o1_ps = psD.tile([64, BPC], f32, name="o1_ps", tag="o1p")
            for h in range(HEADS):
                nc.tensor.matmul(
                    o1_ps[:],
                    ow1_s[:, h * 64 : (h + 1) * 64],
                    fct_bh[:, :, h],
                    start=(h == 0),
                    stop=False,
                    skip_group_check=True,
                )
            nc.tensor.matmul(
                o1_ps[:], ob1_s[:], on16_s[:], start=False, stop=True,
                skip_group_check=True,
            )
            nc.scalar.activation(obuf[0:64, :], o1_ps[:], AF.Relu)
            fin_ps = psD.tile([1, BPC], f32, name="fin_ps", tag="finp")
            nc.tensor.matmul(fin_ps[:], w2o_s[:], obuf[:], start=True, stop=True)
            nc.vector.tensor_copy(out=res[:], in_=fin_ps[:])
            nc.sync.dma_start(out_d.rearrange("(a n) -> a n", a=1), res[:])


def make_in_maps(inputs):
    """Host-side marshalling: slice B across cores and pack weights into the
    layouts the device program expects.

    bf16 note: the big streamed matmuls run in bf16.  To avoid systematic
    model-weight rounding, layer-1 weights are split hi/lo across extra
    contraction rows (w = hi + lo with both bf16); per-point input rounding
    is stochastic and averages out in the softmax pooling."""
    import ml_dtypes

    bf = ml_dtypes.bfloat16
    f = np.float32

    def split(a):
        hi = a.astype(bf)
        lo = (a - hi.astype(f)).astype(bf)
        return hi, lo
    radar = np.concatenate(
        [np.asarray(inputs["radar_xy"], f), np.asarray(inputs["radar_dir"], f)], axis=1
    )  # [B, 4]
    pts = np.asarray(inputs["pts"], f)
    enc_w1 = np.asarray(inputs["enc_w1"], f)
    enc_b1 = np.asarray(inputs["enc_b1"], f)
    enc_w2 = np.asarray(inputs["enc_w2"], f)
    enc_b2 = np.asarray(inputs["enc_b2"], f)
    sc_w1 = np.asarray(inputs["sc_w1"], f)
    sc_b1 = np.asarray(inputs["sc_b1"], f)
    sc_w2 = np.asarray(inputs["sc_w2"], f)
    out_w1 = np.asarray(inputs["out_w1"], f)
    out_b1 = np.asarray(inputs["out_b1"], f)
    out_w2 = np.asarray(inputs["out_w2"], f)
    out_b2 = np.asarray(inputs["out_b2"], f)

    # per-b layer-1 bias vectors (radar is constant over n)
    cb_sc = np.einsum("br,hrd->bhd", radar, sc_w1[:, :4, :]) + sc_b1  # [B, 4, 64]
    cb_enc = radar @ enc_w1[:4] + enc_b1  # [B, 64]

    # xp rows: [xh, yh, xh, yh, 1, 1] (bf16); rows 0-3 feed the weight-split
    # layer-1 matmuls, rows 4-5 carry the (split) bias contraction.
    xp = np.empty((B, 6, N), bf)
    xh = pts[:, :, 0].astype(bf)
    yh = pts[:, :, 1].astype(bf)
    xp[:, 0] = xh
    xp[:, 1] = yh
    xp[:, 2] = xh
    xp[:, 3] = yh
    xp[:, 4] = 1.0
    xp[:, 5] = 1.0

    # Round r of the score layer covers half s=r//4 (heads 2s,2s+1), hidden
    # dims D = 32*(r%4)+d; head hd(r,d) = 2s + D//64, within-head dim D%64.
    def rhd(r, d):
        D = 32 * (r % 4) + d
        return 2 * (r // 4) + D // 64, D % 64

    # w2r: lg lhsT per round: row 32*sl+d, col 4*sl+hd -> logit (slot sl,
    # head hd); cols 16-31 stay zero (pad to a 32-wide col-tile).
    w2r = np.zeros((128, 8 * 32), bf)
    for r in range(8):
        for sl in range(4):
            for d in range(32):
                hd, dm = rhd(r, d)
                w2r[32 * sl + d, r * 32 + 4 * sl + hd] = sc_w2[hd, dm]
    # pooling partial-sum selector: col 4*bi+h sums rows {32*jj+4*bi+h}
    sel4 = np.zeros((128, 16), f)
    for bi in range(4):
        for h in range(4):
            for jj in range(4):
                sel4[32 * jj + 4 * bi + h, 4 * bi + h] = 1.0

    ew2b = np.concatenate([enc_w2, enc_b2[None, :]], axis=0)  # [65, 64]
    ow1 = np.empty((64, 256), f)
    for h in range(HEADS):
        ow1[:, h * 64 : (h + 1) * 64] = out_w1[h * 64 : (h + 1) * 64, :]
    ob1 = np.ascontiguousarray(out_b1[None, :])
    w2o = np.concatenate([out_w2, out_b2[None, :]], axis=0)  # [65, 1]
    id128 = np.eye(128, dtype=bf)
    id64f = np.eye(64, dtype=f)
    on16 = np.ones((1, BPC), f)

    # layer-1 weight splits for wbd
    wxh_s, wxl_s = split(sc_w1[:, 4, :])  # [4, 64] each
    wyh_s, wyl_s = split(sc_w1[:, 5, :])
    cbh_s, cbl_s = split(cb_sc)  # [B, 4, 64]

    in_maps = []
    for c in range(NCORES):
        sl = slice(c * BPC, (c + 1) * BPC)
        # wenm rows [wxh, wyh, wxl, wyl, bh, bl] vs xp rows [xh, yh, xh, yh, 1, 1]
        wenm = np.zeros((6, BPC * 65), bf)
        exh, exl = split(enc_w1[4])
        eyh, eyl = split(enc_w1[5])
        for bl in range(BPC):
            s = slice(bl * 65, bl * 65 + 64)
            wenm[0, s] = exh
            wenm[1, s] = eyh
            wenm[2, s] = exl
            wenm[3, s] = eyl
            bh, blo = split(cb_enc[c * BPC + bl])
            wenm[4, s] = bh
            wenm[5, s] = blo
            wenm[4, bl * 65 + 64] = 1.0
        xpc_core = np.ascontiguousarray(xp[sl])
        # wbd: block-diag score layer-1 lhsT per (round r, group g): rows
        # 32g+4sl+[xh,yh,xl,yl weights], rows 32g+16/17 = hi/lo bias (b=4sl+g)
        wbd = np.zeros((128, 8 * 128), bf)
        for r in range(8):
            hds = np.empty(128, np.int64)
            dms = np.empty(128, np.int64)
            for slt in range(4):
                for d in range(32):
                    hd, dm = rhd(r, d)
                    hds[32 * slt + d] = hd
                    dms[32 * slt + d] = dm
            for g in range(4):
                for slt in range(4):
                    cols = np.arange(r * 128 + 32 * slt, r * 128 + 32 * slt + 32)
                    h_v = hds[32 * slt : 32 * slt + 32]
                    d_v = dms[32 * slt : 32 * slt + 32]
                    wbd[32 * g + 4 * slt + 0, cols] = wxh_s[h_v, d_v]
                    wbd[32 * g + 4 * slt + 1, cols] = wyh_s[h_v, d_v]
                    wbd[32 * g + 4 * slt + 2, cols] = wxl_s[h_v, d_v]
                    wbd[32 * g + 4 * slt + 3, cols] = wyl_s[h_v, d_v]
                    gb = c * BPC + 4 * slt + g
                    wbd[32 * g + 16, cols] = cbh_s[gb, h_v, d_v]
                    wbd[32 * g + 17, cols] = cbl_s[gb, h_v, d_v]
        # xpr: per chunk [128, CHUNK]: rows 32g+4sl+[xh,yh,xh,yh] of b=4sl+g,
        # rows 32g+16/17 = ones (bias contraction)
        xpr = np.zeros((NCH, 128, CHUNK), bf)
        for lb in range(BPC):
            g, slt = lb % 4, lb // 4
            gb = c * BPC + lb
            xb = xh[gb].reshape(NCH, CHUNK)
            yb = yh[gb].reshape(NCH, CHUNK)
            xpr[:, 32 * g + 4 * slt + 0, :] = xb
            xpr[:, 32 * g + 4 * slt + 1, :] = yb
            xpr[:, 32 * g + 4 * slt + 2, :] = xb
            xpr[:, 32 * g + 4 * slt + 3, :] = yb
        for g in range(4):
            xpr[:, 32 * g + 16, :] = 1.0
            xpr[:, 32 * g + 17, :] = 1.0
        in_maps.append(
            dict(
                xp=xpc_core,
                xpr=xpr,
                wbd=wbd,
                w2r=w2r,
                wenm=wenm,
                ew2b=ew2b,
                ow1=ow1,
                ob1=ob1,
                w2o=w2o,
                id128=id128,
                id64f=id64f,
                sel4=sel4,
                on16=on16,
            )
        )
    return in_maps


_CACHE = {}


def _get_runner():
    """Build the Bass program once and a cached jitted PJRT executable over
    the 8 cores (shard_map along axis 0 of every input)."""
    if "runner" in _CACHE:
        return _CACHE["runner"]

    import jax
    from jax.sharding import Mesh, NamedSharding, PartitionSpec

    from concourse.bass2jax import (
        _bass_exec_p,
        install_neuronx_cc_hook,
        partition_id_tensor,
        shard_map,
    )

    nc = build_nc()
    install_neuronx_cc_hook()
    partition_name = nc.partition_id_tensor.name if nc.partition_id_tensor else None
    in_names, out_names, out_avals = [], [], []
    for alloc in nc.m.functions[0].allocations:
        if not isinstance(alloc, mybir.MemoryLocationSet):
            continue
        name = alloc.memorylocations[0].name
        if alloc.kind == "ExternalInput":
            if name != partition_name:
                in_names.append(name)
        elif alloc.kind == "ExternalOutput":
            out_names.append(name)
            out_avals.append(
                jax.core.ShapedArray(tuple(alloc.tensor_shape), mybir.dt.np(alloc.dtype))
            )
    all_in_names = tuple(in_names + out_names)
    if partition_name is not None:
        all_in_names = all_in_names + (partition_name,)

    def _body(*args):
        operands = list(args)
        if partition_name is not None:
            operands.append(partition_id_tensor())
        return tuple(
            _bass_exec_p.bind(
                *operands,
                out_avals=tuple(out_avals),
                in_names=all_in_names,
                out_names=tuple(out_names),
                lowering_input_output_aliases=(),
                sim_require_finite=True,
                sim_require_nnan=True,
                nc=nc,
            )
        )

    devices = jax.devices()[:NCORES]
    mesh = Mesh(np.asarray(devices), ("core",))
    nin = len(in_names) + len(out_names)
    fn = jax.jit(
        shard_map(
            _body,
            mesh=mesh,
            in_specs=(PartitionSpec("core"),) * nin,
            out_specs=(PartitionSpec("core"),) * len(out_names),
            check_rep=False,
        ),
        keep_unused=True,
    )
    sharding = NamedSharding(mesh, PartitionSpec("core"))
    runner = (fn, sharding, in_names, out_avals)
    _CACHE["runner"] = runner
    return runner


def kernel(**inputs):
    import jax

    in_maps = make_in_maps(inputs)
    fn, sharding, in_names, out_avals = _get_runner()
    concat_in = [
        np.concatenate([np.asarray(in_maps[c][name]) for c in range(NCORES)], axis=0)
        for name in in_names
    ]
    concat_zeros = [
        np.zeros((NCORES * a.shape[0], *a.shape[1:]), a.dtype) for a in out_avals
    ]
    args = [jax.device_put(a, sharding) for a in (*concat_in, *concat_zeros)]
    (out,) = fn(*args)
    return np.asarray(out).reshape(B).astype(np.float32)

